# revision 26
# baseline (speedup 1.0000x reference)
"""Trainium2 Bass kernel for nn_ChebyshevLayer_89489938580012.

Math: the reference output depends on x only through its leading 12x12
2-D Chebyshev modes per (batch, patch).  The whole pipeline is linear:

  out[b,p,:,:,o] = G @ T[b,p,o] @ G.T,   G = Finv @ M  (256x256)

where T = M1c @ core @ M1c.T modified only on rows {0,1} / cols {0,1}
(boundary conditions + continuity averaging), M1c = M_1[:, :12], and
core = channel-mixed modes of x.  Every such T lives in span(Bb) x span(Bb)
with Bb = [M1c | I[:, :12]] (256x24), so T = Bb @ W @ Bb.T with W 24x24
per (b, p, out-channel).  Device work is therefore two memory-bound passes:

  pass A (reads x): Y1[b,p,u,(ny,ci)] = sum_nx F12[u,nx] x[b,p,nx,ny,ci]
  host  (tiny): finish mode reduction, channel mix, BC/continuity in W-space;
                upload What (24x24 per b,p,o) with Ub = G @ Bb  (256x24)
  pass B (writes out): H = What @ Ub.T on device (192 small matmuls
                against the resident Ub.T), then out[b,p] = Ub @ H

The DMA wire time is globally serialized in HW, so the floor is total
bytes moved / ~360 GB/s.  x, y1 and out therefore travel as bf16 (host
rounds x before upload; host upcasts out after download), halving the
wire bytes; all device compute accumulates in fp32 PSUM.  DMA issue
alternates between the two HWDGE queues (SP / Activation) so descriptor
generation pipelines under the wire, and PSUM->SBUF copies are split
between the Vector and Activation engines to stay under the wire time.

Sharding: data-parallel over batch, 2 batches (x 3 patches) per core.
"""

import os
import numpy as np
import ml_dtypes

BF16 = ml_dtypes.bfloat16

B, P, NX, NY, CI, CO = 16, 3, 256, 256, 32, 32
MODES = 12
NCORES = 8
BPC = B // NCORES          # batches per core
NBP = BPC * P              # (b,p) pairs per core
FA = NY * CI               # free dim of pass A rows (8192)
FB = NY * CO               # free dim of pass B rows (8192)
R = 24                     # rank of the factored representation

_SIM = os.environ.get("CHEB_SIM", "0") == "1"

# ---------------------------------------------------------------------------
# Host-side constant matrices (derived from DCT-I definitions in the model)
# ---------------------------------------------------------------------------


def _dct_mats(N=NX, dtype=np.float64):
    n = np.arange(N)
    k = np.arange(N)
    C = np.cos(np.pi * np.outer(k, n) / (N - 1))
    w = np.full(N, 2.0)
    w[0] = w[-1] = 1.0
    s = np.ones(N)
    s[0] = s[-1] = 0.5
    F = (s[:, None] * C * w[None, :]) / (N - 1)   # values -> cheb coeffs
    Finv = C.copy()                               # cheb coeffs -> values
    return F.astype(dtype), Finv.astype(dtype)


_F, _FINV = _dct_mats()
_F12 = _F[:MODES, :]                              # (12, 256)


# ---------------------------------------------------------------------------
# Bass programs (built once, reused across calls)
# ---------------------------------------------------------------------------

_PROGS = {}


def _build_pass_a():
    import concourse.tile as tile
    from concourse import bacc, mybir

    nc = bacc.Bacc()
    f32 = mybir.dt.float32
    bf16 = mybir.dt.bfloat16
    x_d = nc.dram_tensor("x", [NBP, NX, FA], bf16, kind="ExternalInput")
    f12t_d = nc.dram_tensor("f12t", [NX, MODES], bf16, kind="ExternalInput")
    y1_d = nc.dram_tensor("y1", [NBP, MODES, FA], bf16, kind="ExternalOutput")

    with tile.TileContext(nc) as tc:
        with tc.tile_pool(name="const", bufs=1) as cpool, \
             tc.tile_pool(name="xin", bufs=2) as xpool, \
             tc.tile_pool(name="ps", bufs=2, space="PSUM") as ppool, \
             tc.tile_pool(name="yout", bufs=2) as ypool:
            f12c = cpool.tile([128, 2 * MODES], bf16, tag="f12c")
            nc.sync.dma_start(out=f12c[:, :MODES], in_=f12t_d[0:128, :])
            nc.scalar.dma_start(out=f12c[:, MODES:], in_=f12t_d[128:256, :])
            f12 = [f12c[:, :MODES], f12c[:, MODES:]]
            qs = [nc.sync, nc.scalar]
            CH = 2048
            for bp in range(NBP):
                ysb = ypool.tile([MODES, FA], bf16)
                for cc in range(FA // CH):              # 2048-wide chunks
                    xts = []
                    for kc in range(2):
                        xt = xpool.tile([128, CH], bf16, tag=f"x{kc}_{cc}")
                        qs[kc].dma_start(
                            out=xt[:],
                            in_=x_d[bp, kc * 128:(kc + 1) * 128,
                                    cc * CH:(cc + 1) * CH])
                        xts.append(xt)
                    ps = ppool.tile([MODES, CH], f32)
                    for sub in range(CH // 512):
                        s = sub * 512
                        for kc in range(2):
                            nc.tensor.matmul(
                                ps[:, s:s + 512], lhsT=f12[kc],
                                rhs=xts[kc][:, s:s + 512],
                                start=(kc == 0), stop=(kc == 1))
                    nc.vector.tensor_copy(
                        out=ysb[:, cc * CH:(cc + 1) * CH], in_=ps[:])
                nc.gpsimd.dma_start(out=y1_d[bp], in_=ysb[:])
    nc.compile()
    return nc


def _build_pass_b():
    import concourse.tile as tile
    from concourse import bacc, mybir

    nc = bacc.Bacc()
    f32 = mybir.dt.float32
    bf16 = mybir.dt.bfloat16
    # h[bp, r, o*NY + y] = H[bp, r, o, y] = sum_s What[bp,o,r,s] Ub[y,s]
    h_d = nc.dram_tensor("h", [NBP, R, CO * NY], bf16, kind="ExternalInput")
    ubt_d = nc.dram_tensor("ubt", [R, NX], bf16, kind="ExternalInput")
    out_d = nc.dram_tensor("out", [NBP, NX, FB], bf16, kind="ExternalOutput")

    with tile.TileContext(nc) as tc:
        with tc.tile_pool(name="const", bufs=1) as cpool, \
             tc.tile_pool(name="hin", bufs=3) as hpool, \
             tc.tile_pool(name="ps", bufs=4, space="PSUM") as ppool, \
             tc.tile_pool(name="osb", bufs=2) as opool:
            ubc = cpool.tile([R, NX], bf16, tag="ubc")
            nc.sync.dma_start(out=ubc[:], in_=ubt_d[:])
            qi = 0
            # copy engines per 1024-col group (PSUM readable by DVE/ACT only)
            cp_engs = [0, 1, 0, 1, 0, 1, 0, 0,     # xc=0: DVE x5, ACT x3
                       0, 1, 0, 1, 0, 1, 0, 1]     # xc=1: DVE x4, ACT x4
            for bp in range(NBP):
                hsb = hpool.tile([R, CO, NY], bf16)
                nc.sync.dma_start(out=hsb[:], in_=h_d[bp])
                # out[x, y*32+o] = sum_r Ub[x, r] H[r, o, y]
                for xc in range(2):
                    osb = opool.tile([128, FB], bf16, tag=f"os{xc}")
                    for cg in range(8):     # 1024 out cols = 32 y x 32 o
                        ps = ppool.tile([128, 1024], f32)
                        for sub in range(2):
                            ch = cg * 2 + sub
                            rhs = hsb[:, :, ch * 16:(ch + 1) * 16].rearrange(
                                "r o y -> r y o")
                            nc.tensor.matmul(
                                ps[:, sub * 512:(sub + 1) * 512],
                                lhsT=ubc[:, xc * 128:(xc + 1) * 128],
                                rhs=rhs, start=True, stop=True)
                        dst = osb[:, cg * 1024:(cg + 1) * 1024]
                        if cp_engs[xc * 8 + cg] == 0:
                            nc.vector.tensor_copy(out=dst, in_=ps[:])
                        else:
                            nc.scalar.copy(out=dst, in_=ps[:])
                        if cg % 2 == 1:     # store each finished 2048-col chunk
                            sq = [nc.gpsimd, nc.sync, nc.gpsimd, nc.scalar,
                                  nc.gpsimd, nc.sync, nc.gpsimd, nc.sync][qi % 8]
                            sq.dma_start(
                                out=out_d[bp, xc * 128:(xc + 1) * 128,
                                          (cg - 1) * 1024:(cg + 1) * 1024],
                                in_=osb[:, (cg - 1) * 1024:(cg + 1) * 1024])
                            qi += 1
    nc.compile()
    return nc


def _get_prog(name):
    if name not in _PROGS:
        _PROGS[name] = _build_pass_a() if name == "a" else _build_pass_b()
    return _PROGS[name]


EXEC_NS = {}
WALL_NS = {}


def _run_spmd(nc, in_maps, out_name, sane_max):
    import time
    from concourse.bass_utils import run_bass_kernel_spmd
    trace = os.environ.get("CHEB_TRACE", "0") == "1"
    t0 = time.perf_counter()
    for attempt in range(3):
        res = run_bass_kernel_spmd(nc, in_maps, list(range(NCORES)),
                                   trace=trace)
        outs = [np.asarray(r[out_name], dtype=np.float32)
                for r in res.results]
        # transient transport glitches show up as huge garbage values
        if all(np.isfinite(o).all() and np.abs(o).max() < sane_max
               for o in outs):
            break
    WALL_NS[out_name] = int((time.perf_counter() - t0) * 1e9)
    if res.exec_time_ns is not None:
        EXEC_NS[out_name] = res.exec_time_ns
    return outs


# ---------------------------------------------------------------------------
# Host middle step: BC + continuity in the 24x24 W-representation
# ---------------------------------------------------------------------------


def _middle(core, M_1):
    """core: (B, P, 12, 12, CO) float64 -> W: (B, P, CO, 24, 24) float64.

    W-representation: T = Bb @ W @ Bb.T with Bb = [M1c | I[:, :12]].
    Row/col index r<12 -> M1c column r; r>=12 -> unit vector e_{r-12}.
    """
    M1c = M_1[:, :MODES].astype(np.float64)          # (256, 12)
    brow = np.zeros((2, R))                          # b_x = Bb[x, :] for x=0,1
    for x0 in range(2):
        brow[x0, :MODES] = M1c[x0]
        brow[x0, MODES + x0] = 1.0
    B12 = np.zeros((MODES, R))                       # Bb[:12, :]
    B12[:, :MODES] = M1c[:MODES]
    B12[np.arange(MODES), MODES + np.arange(MODES)] += 1.0

    W = np.zeros(core.shape[:2] + (CO, R, R))
    W[..., :MODES, :MODES] = np.moveaxis(core, -1, 2)

    def zero_row(p, x0):
        W[:, p, :, MODES + x0, :] -= np.einsum("k,bokl->bol", brow[x0], W[:, p])

    def zero_col(p, y0):
        W[:, p, :, :, MODES + y0] -= np.einsum("bokl,l->bok", W[:, p], brow[y0])

    def read_col12(p, y0):
        return np.einsum("uk,bokl,l->bou", B12, W[:, p], brow[y0])

    def read_row12(p, x0):
        return np.einsum("k,bokl,ul->bou", brow[x0], W[:, p], B12)

    def read_entry(p, x0, y0):
        return np.einsum("k,bokl,l->bo", brow[x0], W[:, p], brow[y0])

    def set_col12(p, y0, v):
        W[:, p, :, MODES:, MODES + y0] += v - read_col12(p, y0)

    def set_row12(p, x0, v):
        W[:, p, :, MODES + x0, MODES:] += v - read_row12(p, x0)

    # Strong_BC zeroing (matches reference order; ops on one patch commute)
    zero_col(0, 0); zero_row(0, 0); zero_row(0, 1)
    zero_col(1, 1); zero_row(1, 0)
    zero_row(2, 1); zero_col(2, 0); zero_col(2, 1)

    # Continuity averaging
    tmp1 = 0.5 * (read_col12(0, 1) + read_col12(1, 0))       # (B, CO, 12)
    tmp2 = 0.5 * (read_row12(2, 0) + read_row12(1, 1))
    tmp12 = (read_entry(0, 1, 1) + read_entry(1, 1, 0)
             + read_entry(2, 0, 0)) / 3.0
    tmp1[:, :, 1] = tmp12
    tmp2[:, :, 0] = tmp12
    set_col12(0, 1, tmp1)
    set_col12(1, 0, tmp1)
    set_row12(2, 0, tmp2)
    set_row12(1, 1, tmp2)
    return W


# ---------------------------------------------------------------------------
# Top-level kernel
# ---------------------------------------------------------------------------


def kernel(x, weights, M, M_1):
    x = np.asarray(x, dtype=np.float32)
    weights = np.asarray(weights, dtype=np.float32)
    M = np.asarray(M, dtype=np.float64)
    M_1 = np.asarray(M_1, dtype=np.float64)

    # ---- pass A: x -> Y1 (contract nx with F12) ----------------------------
    xr = np.ascontiguousarray(x.reshape(B, P, NX, FA)).astype(BF16)
    f12t = np.ascontiguousarray(_F12.T).astype(BF16)         # (256, 12)
    if _SIM:
        y1 = np.einsum("un,bpnf->bpuf", _F12.astype(np.float32),
                       xr.astype(np.float32))
    else:
        in_maps = [{"x": np.ascontiguousarray(
                        xr[c * BPC:(c + 1) * BPC].reshape(NBP, NX, FA)),
                    "f12t": f12t} for c in range(NCORES)]
        outs = _run_spmd(_get_prog("a"), in_maps, "y1", 1e3)
        y1 = np.concatenate(
            [o.reshape(BPC, P, MODES, FA) for o in outs], 0)

    # ---- host: finish reduction + channel mix + BC/continuity --------------
    y1 = y1.reshape(B, P, MODES, NY, CI).astype(np.float64)
    z = np.einsum("vn,bpuni->bpuvi", _F12, y1)               # (B,P,12,12,CI)
    core = np.einsum("bpuvi,uvio->bpuvo", z, weights.astype(np.float64))
    W = _middle(core, M_1)                                   # (B,P,CO,24,24)

    G = _FINV @ M                                            # (256, 256)
    Bb = np.zeros((NX, R))
    Bb[:, :MODES] = M_1[:, :MODES]
    Bb[np.arange(MODES), MODES + np.arange(MODES)] += 1.0
    Ub = G @ Bb                                              # (256, 24)

    # host computes H = What @ Ub.T (small), device only does out = Ub @ H
    H = np.einsum("bpors,ys->bproy", W, Ub)                  # (B,P,R,CO,NY)
    h16 = np.ascontiguousarray(H.reshape(B, P, R, CO * NY)).astype(BF16)

    # ---- pass B: out = Ub @ H ----------------------------------------------
    ubt = np.ascontiguousarray(Ub.T).astype(BF16)            # (24, 256)
    if _SIM:
        out = np.einsum("xr,bproy->bpxyo", Ub,
                        h16.astype(np.float64).reshape(B, P, R, CO, NY))
        out = out.reshape(B, P, NX, FB).astype(np.float32).astype(BF16)
    else:
        in_maps = [{"h": np.ascontiguousarray(
                        h16[c * BPC:(c + 1) * BPC].reshape(NBP, R, CO * NY)),
                    "ubt": ubt} for c in range(NCORES)]
        outs = _run_spmd(_get_prog("b"), in_maps, "out", 1e3)
        out = np.concatenate(
            [o.reshape(BPC, P, NX, FB) for o in outs], 0)

    return np.ascontiguousarray(
        out.astype(np.float32).reshape(B, P, NX, NY, CO))


# revision 28
# speedup vs baseline: 1.0335x; 1.0335x over previous
"""Trainium2 Bass kernel for nn_ChebyshevLayer_89489938580012.

Math: the reference output depends on x only through its leading 12x12
2-D Chebyshev modes per (batch, patch).  The whole pipeline is linear:

  out[b,p,:,:,o] = G @ T[b,p,o] @ G.T,   G = Finv @ M  (256x256)

where T = M1c @ core @ M1c.T modified only on rows {0,1} / cols {0,1}
(boundary conditions + continuity averaging), M1c = M_1[:, :12], and
core = channel-mixed modes of x.  Every such T lives in span(Bb) x span(Bb)
with Bb = [M1c | I[:, :12]] (256x24), so T = Bb @ W @ Bb.T with W 24x24
per (b, p, out-channel).  Device work is therefore two memory-bound passes:

  pass A (reads x): Y1[b,p,u,(ny,ci)] = sum_nx F12[u,nx] x[b,p,nx,ny,ci]
  host  (tiny): finish mode reduction, channel mix, BC/continuity in W-space;
                upload What (24x24 per b,p,o) with Ub = G @ Bb  (256x24)
  pass B (writes out): H = What @ Ub.T on device (192 small matmuls
                against the resident Ub.T), then out[b,p] = Ub @ H

The DMA wire time is globally serialized in HW, so the floor is total
bytes moved / ~360 GB/s.  x, y1 and out therefore travel as bf16 (host
rounds x before upload; host upcasts out after download), halving the
wire bytes; all device compute accumulates in fp32 PSUM.  DMA issue
alternates between the two HWDGE queues (SP / Activation) so descriptor
generation pipelines under the wire, and PSUM->SBUF copies are split
between the Vector and Activation engines to stay under the wire time.

Sharding: data-parallel over batch, 2 batches (x 3 patches) per core.
"""

import os
import numpy as np
import ml_dtypes

BF16 = ml_dtypes.bfloat16

B, P, NX, NY, CI, CO = 16, 3, 256, 256, 32, 32
MODES = 12
NCORES = 8
BPC = B // NCORES          # batches per core
NBP = BPC * P              # (b,p) pairs per core
FA = NY * CI               # free dim of pass A rows (8192)
FB = NY * CO               # free dim of pass B rows (8192)
R = 24                     # rank of the factored representation

_SIM = os.environ.get("CHEB_SIM", "0") == "1"

# ---------------------------------------------------------------------------
# Host-side constant matrices (derived from DCT-I definitions in the model)
# ---------------------------------------------------------------------------


def _dct_mats(N=NX, dtype=np.float64):
    n = np.arange(N)
    k = np.arange(N)
    C = np.cos(np.pi * np.outer(k, n) / (N - 1))
    w = np.full(N, 2.0)
    w[0] = w[-1] = 1.0
    s = np.ones(N)
    s[0] = s[-1] = 0.5
    F = (s[:, None] * C * w[None, :]) / (N - 1)   # values -> cheb coeffs
    Finv = C.copy()                               # cheb coeffs -> values
    return F.astype(dtype), Finv.astype(dtype)


_F, _FINV = _dct_mats()
_F12 = _F[:MODES, :]                              # (12, 256)


# ---------------------------------------------------------------------------
# Bass programs (built once, reused across calls)
# ---------------------------------------------------------------------------

_PROGS = {}


def _build_pass_a():
    import concourse.tile as tile
    from concourse import bacc, mybir

    nc = bacc.Bacc()
    f32 = mybir.dt.float32
    bf16 = mybir.dt.bfloat16
    x_d = nc.dram_tensor("x", [NBP, NX, FA], bf16, kind="ExternalInput")
    f12t_d = nc.dram_tensor("f12t", [NX, MODES], bf16, kind="ExternalInput")
    y1_d = nc.dram_tensor("y1", [NBP, MODES, FA], bf16, kind="ExternalOutput")

    with tile.TileContext(nc) as tc:
        with tc.tile_pool(name="const", bufs=1) as cpool, \
             tc.tile_pool(name="xin", bufs=2) as xpool, \
             tc.tile_pool(name="ps", bufs=2, space="PSUM") as ppool, \
             tc.tile_pool(name="yout", bufs=2) as ypool:
            f12c = cpool.tile([128, 2 * MODES], bf16, tag="f12c")
            nc.sync.dma_start(out=f12c[:, :MODES], in_=f12t_d[0:128, :])
            nc.scalar.dma_start(out=f12c[:, MODES:], in_=f12t_d[128:256, :])
            f12 = [f12c[:, :MODES], f12c[:, MODES:]]
            qs = [nc.sync, nc.scalar]
            CH = 2048
            for bp in range(NBP):
                ysb = ypool.tile([MODES, FA], bf16)
                for cc in range(FA // CH):              # 2048-wide chunks
                    xts = []
                    for kc in range(2):
                        xt = xpool.tile([128, CH], bf16, tag=f"x{kc}_{cc}")
                        qs[kc].dma_start(
                            out=xt[:],
                            in_=x_d[bp, kc * 128:(kc + 1) * 128,
                                    cc * CH:(cc + 1) * CH])
                        xts.append(xt)
                    ps = ppool.tile([MODES, CH], f32)
                    for sub in range(CH // 512):
                        s = sub * 512
                        for kc in range(2):
                            nc.tensor.matmul(
                                ps[:, s:s + 512], lhsT=f12[kc],
                                rhs=xts[kc][:, s:s + 512],
                                start=(kc == 0), stop=(kc == 1))
                    nc.vector.tensor_copy(
                        out=ysb[:, cc * CH:(cc + 1) * CH], in_=ps[:])
                nc.gpsimd.dma_start(out=y1_d[bp], in_=ysb[:])
    nc.compile()
    return nc


def _build_pass_b():
    import concourse.tile as tile
    from concourse import bacc, mybir

    nc = bacc.Bacc()
    f32 = mybir.dt.float32
    bf16 = mybir.dt.bfloat16
    # h[bp, r, o*NY + y] = H[bp, r, o, y] = sum_s What[bp,o,r,s] Ub[y,s]
    h_d = nc.dram_tensor("h", [NBP, R, CO * NY], bf16, kind="ExternalInput")
    ubt_d = nc.dram_tensor("ubt", [R, NX], bf16, kind="ExternalInput")
    out_d = nc.dram_tensor("out", [NBP, NX, FB], bf16, kind="ExternalOutput")

    with tile.TileContext(nc) as tc:
        with tc.tile_pool(name="const", bufs=1) as cpool, \
             tc.tile_pool(name="hin", bufs=3) as hpool, \
             tc.tile_pool(name="ps", bufs=4, space="PSUM") as ppool, \
             tc.tile_pool(name="osb", bufs=2) as opool:
            ubc = cpool.tile([R, NX], bf16, tag="ubc")
            nc.sync.dma_start(out=ubc[:], in_=ubt_d[:])
            qi = 0
            # copy engines per 1024-col group (PSUM readable by DVE/ACT only)
            cp_engs = [1, 0, 1, 0, 1, 0, 1, 0,
                       1, 0, 1, 0, 1, 0, 1, 0]     # strict ACT/DVE alternation
            for bp in range(NBP):
                hsb = hpool.tile([R, CO, NY], bf16)
                nc.sync.dma_start(out=hsb[:], in_=h_d[bp])
                # out[x, y*32+o] = sum_r Ub[x, r] H[r, o, y]
                for xc in range(2):
                    osb = opool.tile([128, FB], bf16, tag=f"os{xc}")
                    for cg in range(8):     # 1024 out cols = 32 y x 32 o
                        ps = ppool.tile([128, 1024], f32)
                        for sub in range(2):
                            ch = cg * 2 + sub
                            rhs = hsb[:, :, ch * 16:(ch + 1) * 16].rearrange(
                                "r o y -> r y o")
                            nc.tensor.matmul(
                                ps[:, sub * 512:(sub + 1) * 512],
                                lhsT=ubc[:, xc * 128:(xc + 1) * 128],
                                rhs=rhs, start=True, stop=True)
                        dst = osb[:, cg * 1024:(cg + 1) * 1024]
                        if cp_engs[xc * 8 + cg] == 0:
                            nc.vector.tensor_copy(out=dst, in_=ps[:])
                        else:
                            nc.scalar.copy(out=dst, in_=ps[:])
                        if cg % 2 == 1:     # store each finished 2048-col chunk
                            sq = [nc.gpsimd, nc.sync, nc.gpsimd, nc.scalar,
                                  nc.gpsimd, nc.sync, nc.gpsimd, nc.sync][qi % 8]
                            sq.dma_start(
                                out=out_d[bp, xc * 128:(xc + 1) * 128,
                                          (cg - 1) * 1024:(cg + 1) * 1024],
                                in_=osb[:, (cg - 1) * 1024:(cg + 1) * 1024])
                            qi += 1
    nc.compile()
    return nc


def _get_prog(name):
    if name not in _PROGS:
        _PROGS[name] = _build_pass_a() if name == "a" else _build_pass_b()
    return _PROGS[name]


EXEC_NS = {}
WALL_NS = {}


def _run_spmd(nc, in_maps, out_name, sane_max):
    import time
    from concourse.bass_utils import run_bass_kernel_spmd
    trace = os.environ.get("CHEB_TRACE", "0") == "1"
    t0 = time.perf_counter()
    for attempt in range(3):
        res = run_bass_kernel_spmd(nc, in_maps, list(range(NCORES)),
                                   trace=trace)
        outs = [np.asarray(r[out_name], dtype=np.float32)
                for r in res.results]
        # transient transport glitches show up as huge garbage values
        if all(np.isfinite(o).all() and np.abs(o).max() < sane_max
               for o in outs):
            break
    WALL_NS[out_name] = int((time.perf_counter() - t0) * 1e9)
    if res.exec_time_ns is not None:
        EXEC_NS[out_name] = res.exec_time_ns
    return outs


# ---------------------------------------------------------------------------
# Host middle step: BC + continuity in the 24x24 W-representation
# ---------------------------------------------------------------------------


def _middle(core, M_1):
    """core: (B, P, 12, 12, CO) float64 -> W: (B, P, CO, 24, 24) float64.

    W-representation: T = Bb @ W @ Bb.T with Bb = [M1c | I[:, :12]].
    Row/col index r<12 -> M1c column r; r>=12 -> unit vector e_{r-12}.
    """
    M1c = M_1[:, :MODES].astype(np.float64)          # (256, 12)
    brow = np.zeros((2, R))                          # b_x = Bb[x, :] for x=0,1
    for x0 in range(2):
        brow[x0, :MODES] = M1c[x0]
        brow[x0, MODES + x0] = 1.0
    B12 = np.zeros((MODES, R))                       # Bb[:12, :]
    B12[:, :MODES] = M1c[:MODES]
    B12[np.arange(MODES), MODES + np.arange(MODES)] += 1.0

    W = np.zeros(core.shape[:2] + (CO, R, R))
    W[..., :MODES, :MODES] = np.moveaxis(core, -1, 2)

    def zero_row(p, x0):
        W[:, p, :, MODES + x0, :] -= np.einsum("k,bokl->bol", brow[x0], W[:, p])

    def zero_col(p, y0):
        W[:, p, :, :, MODES + y0] -= np.einsum("bokl,l->bok", W[:, p], brow[y0])

    def read_col12(p, y0):
        return np.einsum("uk,bokl,l->bou", B12, W[:, p], brow[y0])

    def read_row12(p, x0):
        return np.einsum("k,bokl,ul->bou", brow[x0], W[:, p], B12)

    def read_entry(p, x0, y0):
        return np.einsum("k,bokl,l->bo", brow[x0], W[:, p], brow[y0])

    def set_col12(p, y0, v):
        W[:, p, :, MODES:, MODES + y0] += v - read_col12(p, y0)

    def set_row12(p, x0, v):
        W[:, p, :, MODES + x0, MODES:] += v - read_row12(p, x0)

    # Strong_BC zeroing (matches reference order; ops on one patch commute)
    zero_col(0, 0); zero_row(0, 0); zero_row(0, 1)
    zero_col(1, 1); zero_row(1, 0)
    zero_row(2, 1); zero_col(2, 0); zero_col(2, 1)

    # Continuity averaging
    tmp1 = 0.5 * (read_col12(0, 1) + read_col12(1, 0))       # (B, CO, 12)
    tmp2 = 0.5 * (read_row12(2, 0) + read_row12(1, 1))
    tmp12 = (read_entry(0, 1, 1) + read_entry(1, 1, 0)
             + read_entry(2, 0, 0)) / 3.0
    tmp1[:, :, 1] = tmp12
    tmp2[:, :, 0] = tmp12
    set_col12(0, 1, tmp1)
    set_col12(1, 0, tmp1)
    set_row12(2, 0, tmp2)
    set_row12(1, 1, tmp2)
    return W


# ---------------------------------------------------------------------------
# Top-level kernel
# ---------------------------------------------------------------------------


def kernel(x, weights, M, M_1):
    x = np.asarray(x, dtype=np.float32)
    weights = np.asarray(weights, dtype=np.float32)
    M = np.asarray(M, dtype=np.float64)
    M_1 = np.asarray(M_1, dtype=np.float64)

    # ---- pass A: x -> Y1 (contract nx with F12) ----------------------------
    xr = np.ascontiguousarray(x.reshape(B, P, NX, FA)).astype(BF16)
    f12t = np.ascontiguousarray(_F12.T).astype(BF16)         # (256, 12)
    if _SIM:
        y1 = np.einsum("un,bpnf->bpuf", _F12.astype(np.float32),
                       xr.astype(np.float32))
    else:
        in_maps = [{"x": np.ascontiguousarray(
                        xr[c * BPC:(c + 1) * BPC].reshape(NBP, NX, FA)),
                    "f12t": f12t} for c in range(NCORES)]
        outs = _run_spmd(_get_prog("a"), in_maps, "y1", 1e3)
        y1 = np.concatenate(
            [o.reshape(BPC, P, MODES, FA) for o in outs], 0)

    # ---- host: finish reduction + channel mix + BC/continuity --------------
    y1 = y1.reshape(B, P, MODES, NY, CI).astype(np.float64)
    z = np.einsum("vn,bpuni->bpuvi", _F12, y1)               # (B,P,12,12,CI)
    core = np.einsum("bpuvi,uvio->bpuvo", z, weights.astype(np.float64))
    W = _middle(core, M_1)                                   # (B,P,CO,24,24)

    G = _FINV @ M                                            # (256, 256)
    Bb = np.zeros((NX, R))
    Bb[:, :MODES] = M_1[:, :MODES]
    Bb[np.arange(MODES), MODES + np.arange(MODES)] += 1.0
    Ub = G @ Bb                                              # (256, 24)

    # host computes H = What @ Ub.T (small), device only does out = Ub @ H
    H = np.einsum("bpors,ys->bproy", W, Ub)                  # (B,P,R,CO,NY)
    h16 = np.ascontiguousarray(H.reshape(B, P, R, CO * NY)).astype(BF16)

    # ---- pass B: out = Ub @ H ----------------------------------------------
    ubt = np.ascontiguousarray(Ub.T).astype(BF16)            # (24, 256)
    if _SIM:
        out = np.einsum("xr,bproy->bpxyo", Ub,
                        h16.astype(np.float64).reshape(B, P, R, CO, NY))
        out = out.reshape(B, P, NX, FB).astype(np.float32).astype(BF16)
    else:
        in_maps = [{"h": np.ascontiguousarray(
                        h16[c * BPC:(c + 1) * BPC].reshape(NBP, R, CO * NY)),
                    "ubt": ubt} for c in range(NCORES)]
        outs = _run_spmd(_get_prog("b"), in_maps, "out", 1e3)
        out = np.concatenate(
            [o.reshape(BPC, P, NX, FB) for o in outs], 0)

    return np.ascontiguousarray(
        out.astype(np.float32).reshape(B, P, NX, NY, CO))
